# revision 4
# baseline (speedup 1.0000x reference)
"""Trainium2 Bass kernel for nn_PredictAverageReward.

Pipeline (per core, fruits sharded 8 ways):
  1. GEMM chain on TensorE:  pmtT = M_tool^T-form @ toolsT
                             WT   = M-form @ pmtT
                             pmfT = M_fruit-form @ G^T
                             Rd   = pmfT-form @ WT          [512 fruits, 256 tools]
     (min_r dropped: it cancels in all comparisons; output depends only on
      comparisons.)
  2. Pairwise-win counting: for each column j, compare Rd[:, i] >= Rd[:, j]
     (i < j) with one DVE tensor_scalar (per-partition scalar = column j) or
     ScalarE Sign activation, then reduce over fruits (partition axis) with a
     TensorE matmul whose stationary operand is a one-hot column that routes
     the sums into PSUM row j%128.  All 255 count rows accumulate into just
     two PSUM banks, drained with two copies at the end.
  3. Host: sum per-core counts, threshold at 2048, build the one-hot proposal.
"""

import sys

for _p in ("/opt/trn_rl_repo",):
    if _p not in sys.path:
        sys.path.insert(0, _p)

import numpy as np

import concourse.bass as bass
import concourse.bacc as bacc
import concourse.mybir as mybir
import concourse.tile as tile
from concourse.bass_utils import run_bass_kernel_spmd

F32 = mybir.dt.float32

N_CORES = 8
N_FRUITS, N_TOOLS, P_F, P_T, D = 8192, 1024, 128, 128, 512
K_DOMAIN, BATCH = 256, 4096
F_PER_CORE = BATCH // N_CORES          # 512 fruits per core
N_FT = F_PER_CORE // 128               # 4 fruit partition-tiles

# j >= T_ACT runs on ScalarE (Sign -> {-1,0,1}); j < T_ACT on VectorE (is_ge -> {0,1})
T_ACT = 208

TRACE = False
LAST_RESULTS = None  # BassKernelResults of the last run (for test harness)


def _build_nc():
    nc = bacc.Bacc()

    g_t = nc.dram_tensor("g_t", [128, F_PER_CORE], F32, kind="ExternalInput")
    tools_t = nc.dram_tensor("tools_t", [128, K_DOMAIN], F32, kind="ExternalInput")
    mf = nc.dram_tensor("mf", [P_F, D], F32, kind="ExternalInput")
    mt = nc.dram_tensor("mt", [P_T, D], F32, kind="ExternalInput")
    m_in = nc.dram_tensor("m_in", [D, D], F32, kind="ExternalInput")
    counts = nc.dram_tensor("counts", [128, 2 * K_DOMAIN], F32, kind="ExternalOutput")

    K = K_DOMAIN

    with tile.TileContext(nc) as tc:
        with (
            tc.tile_pool(name="persist", bufs=1) as pp,
            tc.tile_pool(name="ge", bufs=16) as gep,
            tc.tile_pool(name="gemm_ps", bufs=3, space=bass.MemorySpace.PSUM) as gps,
            tc.tile_pool(name="cnt_ps", bufs=2, space=bass.MemorySpace.PSUM) as cps,
        ):
            # ---- constants ----
            zeros = pp.tile([128, K], F32, tag="zeros")
            nc.gpsimd.memset(zeros[:], 0.0)
            # strip[:, 127] = 1, else 0;  strip[:, 127-jj : 255-jj] is a
            # [128, 128] matrix with a single all-ones column at position jj.
            strip = pp.tile([128, 255], F32, tag="strip")
            nc.gpsimd.memset(strip[:], 0.0)
            nc.gpsimd.memset(strip[:, 127:128], 1.0)

            # ---- load inputs ----
            gt_sb = pp.tile([128, F_PER_CORE], F32, tag="gt")
            nc.sync.dma_start(gt_sb[:], g_t[:])
            tools_sb = pp.tile([128, K], F32, tag="tools")
            nc.sync.dma_start(tools_sb[:], tools_t[:])
            mf_sb = pp.tile([128, D], F32, tag="mf")
            nc.sync.dma_start(mf_sb[:], mf[:])
            mt_sb = pp.tile([128, D], F32, tag="mt")
            nc.sync.dma_start(mt_sb[:], mt[:])
            m_sb = []
            for et in range(4):
                t = pp.tile([128, D], F32, tag=f"m{et}")
                nc.sync.dma_start(t[:], m_in[et * 128:(et + 1) * 128, :])
                m_sb.append(t)

            # ---- GEMM chain ----
            # pmtT[e, i] = sum_c M_tool[c, e] * toolsT[c, i]
            pmtT_sb = []
            for et in range(4):
                ps = gps.tile([128, K], F32)
                nc.tensor.matmul(ps[:], mt_sb[:, et * 128:(et + 1) * 128],
                                 tools_sb[:], start=True, stop=True)
                t = pp.tile([128, K], F32, tag=f"pmtT{et}")
                nc.scalar.copy(t[:], ps[:])
                pmtT_sb.append(t)

            # WT[d, i] = sum_e M[e, d] * pmtT[e, i]
            wt_sb = []
            for dt in range(4):
                ps = gps.tile([128, K], F32)
                for et in range(4):
                    nc.tensor.matmul(ps[:], m_sb[et][:, dt * 128:(dt + 1) * 128],
                                     pmtT_sb[et][:], start=(et == 0), stop=(et == 3))
                t = pp.tile([128, K], F32, tag=f"wt{dt}")
                nc.scalar.copy(t[:], ps[:])
                wt_sb.append(t)

            # pmfT[d, f] = sum_c M_fruit[c, d] * G^T[c, f]
            pmfT_sb = []
            for dt in range(4):
                ps = gps.tile([128, F_PER_CORE], F32)
                nc.tensor.matmul(ps[:], mf_sb[:, dt * 128:(dt + 1) * 128],
                                 gt_sb[:], start=True, stop=True)
                t = pp.tile([128, F_PER_CORE], F32, tag=f"pmfT{dt}")
                nc.scalar.copy(t[:], ps[:])
                pmfT_sb.append(t)

            # Rd[f, i] = sum_d pmfT[d, f] * WT[d, i]   (fruits on partitions)
            rd_sb = []
            neg_sb = []
            for ft in range(N_FT):
                ps = gps.tile([128, K], F32)
                for dt in range(4):
                    nc.tensor.matmul(ps[:], pmfT_sb[dt][:, ft * 128:(ft + 1) * 128],
                                     wt_sb[dt][:], start=(dt == 0), stop=(dt == 3))
                t = pp.tile([128, K], F32, tag=f"rd{ft}")
                nc.scalar.copy(t[:], ps[:])
                rd_sb.append(t)
                n = pp.tile([128, K], F32, tag=f"neg{ft}")
                nc.scalar.mul(n[:], ps[:], -1.0)
                neg_sb.append(n)

            # ---- pairwise-win counting ----
            cntA = cps.tile([128, K], F32)   # j in [0, 128)   -> row j
            cntB = cps.tile([128, K], F32)   # j in [128, 256) -> row j-128
            # open one accumulation group per bank; zero-fill data +
            # set has_written for the region so everything below accumulates.
            nc.tensor.matmul(cntA[:], zeros[:, 0:128], zeros[:], start=True, stop=False)
            nc.tensor.matmul(cntB[:], zeros[:, 0:128], zeros[:], start=True, stop=False)

            order = list(range(1, 256))
            lastA = max(j for j in order if j < 128)
            lastB = max(j for j in order if j >= 128)
            for j in order:
                L = j + (j & 1)          # even free-dim keeps DVE in 2x mode
                jj = j % 128
                dst = cntA if j < 128 else cntB
                is_last = (j == (lastA if j < 128 else lastB))
                for ft in range(N_FT):
                    ge = gep.tile([128, K], F32, tag="ge")
                    if j < T_ACT:
                        nc.vector.tensor_scalar(
                            ge[:, 0:L], rd_sb[ft][:, 0:L],
                            rd_sb[ft][:, j:j + 1], None, mybir.AluOpType.is_ge)
                    else:
                        nc.scalar.activation(
                            ge[:, 0:L], rd_sb[ft][:, 0:L],
                            mybir.ActivationFunctionType.Sign,
                            bias=neg_sb[ft][:, j:j + 1], scale=1.0)
                    nc.tensor.matmul(
                        dst[:, 0:L], strip[:, 127 - jj:255 - jj], ge[:, 0:L],
                        start=False, stop=(is_last and ft == N_FT - 1))

            out_sb = pp.tile([128, 2 * K], F32, tag="out")
            nc.scalar.copy(out_sb[:, 0:K], cntA[:])
            nc.vector.tensor_copy(out_sb[:, K:2 * K], cntB[:])
            nc.sync.dma_start(counts[:], out_sb[:])

    nc.compile()
    return nc


def _host_inputs(fruits_prop, tools_prop, M_fruit, M_tool, M, domain_f, domain_t):
    G = np.ascontiguousarray(np.asarray(fruits_prop, np.float32)[np.asarray(domain_f, np.int64)])
    toolsD = np.ascontiguousarray(np.asarray(tools_prop, np.float32)[np.asarray(domain_t, np.int64)])
    toolsD_T = np.ascontiguousarray(toolsD.T)
    mf = np.ascontiguousarray(np.asarray(M_fruit, np.float32))
    mt = np.ascontiguousarray(np.asarray(M_tool, np.float32))
    m = np.ascontiguousarray(np.asarray(M, np.float32))
    in_maps = []
    for c in range(N_CORES):
        Gc = G[c * F_PER_CORE:(c + 1) * F_PER_CORE]
        in_maps.append({
            "g_t": np.ascontiguousarray(Gc.T),
            "tools_t": toolsD_T,
            "mf": mf,
            "mt": mt,
            "m_in": m,
        })
    return in_maps


def decode_counts(counts_sum):
    """counts_sum: [128, 512] summed over cores -> C[i, j] win counts (i < j)."""
    C = np.zeros((K_DOMAIN, K_DOMAIN), np.float64)
    for j in range(1, K_DOMAIN):
        half = j // 128
        row = counts_sum[j % 128, half * K_DOMAIN: half * K_DOMAIN + j]
        if j >= T_ACT:
            C[:j, j] = np.rint((row + BATCH) / 2.0)
        else:
            C[:j, j] = np.rint(row)
    return C


def _predict(C, domain_t, tools_labels):
    pos = np.full(N_TOOLS, -1, np.int64)
    pos[np.asarray(domain_t, np.int64)] = np.arange(K_DOMAIN)
    l1 = pos[np.asarray(tools_labels[0], np.int64)]
    l2 = pos[np.asarray(tools_labels[1], np.int64)]
    B = l1.shape[0]
    half = BATCH // 2
    choice = np.ones(B, np.int64)
    lt = (l1 >= 0) & (l2 >= 0) & (l1 < l2)
    gt = (l1 >= 0) & (l2 >= 0) & (l1 > l2)
    choice[lt] = np.where(C[l1[lt], l2[lt]] >= half, 0, 1)
    choice[gt] = np.where(C[l2[gt], l1[gt]] <= half, 0, 1)
    out = np.zeros((B, 4), np.float32)
    out[:, 0] = 1.0
    out[np.arange(B), 1 + choice] = 1.0
    return out


def kernel(fruits_prop, tools_prop, M_fruit, M_tool, M, min_r, domain_f,
           domain_t, tools_labels):
    global LAST_RESULTS
    in_maps = _host_inputs(fruits_prop, tools_prop, M_fruit, M_tool, M,
                           domain_f, domain_t)
    nc = _build_nc()
    res = run_bass_kernel_spmd(nc, in_maps, list(range(N_CORES)), trace=TRACE)
    LAST_RESULTS = res
    counts_sum = np.zeros((128, 2 * K_DOMAIN), np.float64)
    for c in range(N_CORES):
        counts_sum += res.results[c]["counts"].astype(np.float64)
    C = decode_counts(counts_sum)
    return _predict(C, domain_t, tools_labels)
